# revision 1
# baseline (speedup 1.0000x reference)
"""Self-contained Trainium2 Bass kernel for nn_AdvancedGenuineTransformer_35485019799944.

kernel(**inputs) -> np.ndarray  (full [16,128,32000] logits)
"""


import sys

sys.path.insert(0, "/opt/trn_rl_repo")

from contextlib import ExitStack

import numpy as np

import concourse.bacc as bacc
import concourse.mybir as mybir
import concourse.tile as tile
from concourse.masks import make_identity

FP = mybir.dt.float32
FPR = mybir.dt.float32r
AF = mybir.ActivationFunctionType
OP = mybir.AluOpType
AX = mybir.AxisListType

D = 1024
T = 256          # tokens per core (2 batches x 128)
KT = 8           # d tiles
H = 16
HD = 64
DFF = 4096
V = 32000
INV_SCALE = 0.125          # 1/sqrt(64)
NLOG2E = -1.4426950408889634


def _mm(nc, ps, lhsT, rhs, start, stop, mmdt, tile_position=None):
    nc.tensor.matmul(ps, lhsT.bitcast(mmdt), rhs.bitcast(mmdt),
                     start=start, stop=stop, tile_position=tile_position)


def _register_consts(nc, values):
    for v in values:
        t = nc.alloc_sbuf_tensor(f"const-float32-{v}", [128, 1], FP)
        nc.gpsimd.memset(t.ap(), v)
        nc.const_aps.aps[(FP, v)] = t.ap()
    nc.all_engine_barrier()


def layernorm(nc, pools, hT, g_sl, b_sl, xln, ones_col, mmdt):
    """xln[:,k,:] = ((hT - mean)/sqrt(var+eps)) * g + b   (stats over d)"""
    psst, small1, sqp, bc = (pools["psst"], pools["small1"], pools["lnsq"],
                             pools["lnbc"])

    ps_s = psst.tile([1, T], FP, tag="psacc")
    ps_q = psst.tile([1, T], FP, tag="psacc")
    for k in range(KT):
        h32r = sqp.tile([128, T], FPR, tag="lnsq")
        nc.vector.tensor_copy(h32r[:, :], hT[:, k, :])
        _mm(nc, ps_s[:, :], ones_col[:, :1], h32r[:, :], k == 0, k == KT - 1, mmdt)
    for k in range(KT):
        sq = sqp.tile([128, T], FPR, tag="lnsq")
        nc.scalar.activation(sq[:, :], hT[:, k, :], AF.Square)
        _mm(nc, ps_q[:, :], ones_col[:, :1], sq[:, :], k == 0, k == KT - 1, mmdt)

    mean = small1.tile([1, T], FP, tag="sm1")
    nc.scalar.mul(mean[:, :], ps_s[:, :], 1.0 / D)
    msq = small1.tile([1, T], FP, tag="sm1")
    nc.scalar.mul(msq[:, :], ps_q[:, :], 1.0 / D)
    m2 = small1.tile([1, T], FP, tag="sm1")
    nc.vector.tensor_tensor(out=m2[:, :], in0=mean[:, :], in1=mean[:, :], op=OP.mult)
    var = small1.tile([1, T], FP, tag="sm1")
    nc.vector.tensor_tensor(out=var[:, :], in0=msq[:, :], in1=m2[:, :],
                            op=OP.subtract)
    std = small1.tile([1, T], FP, tag="sm1")
    nc.scalar.activation(std[:, :], var[:, :], AF.Sqrt, bias=1e-5)
    rs = small1.tile([1, T], FP, tag="sm1")
    nc.vector.reciprocal(rs[:, :], std[:, :])
    nm = small1.tile([1, T], FP, tag="sm1")
    nc.vector.scalar_tensor_tensor(out=nm[:, :], in0=mean[:, :], scalar=-1.0,
                                   in1=rs[:, :], op0=OP.mult, op1=OP.mult)
    A_b = bc.tile([128, T], FP, tag="lnbc")
    B_b = bc.tile([128, T], FP, tag="lnbc")
    nc.gpsimd.partition_broadcast(A_b[:, :], rs[:, :])
    nc.gpsimd.partition_broadcast(B_b[:, :], nm[:, :])

    for k in range(KT):
        t1 = sqp.tile([128, T], FP, tag="lnsq")
        nc.vector.tensor_tensor(out=t1[:, :], in0=hT[:, k, :], in1=A_b[:, :],
                                op=OP.mult)
        t2 = sqp.tile([128, T], FP, tag="lnsq")
        nc.vector.tensor_tensor(out=t2[:, :], in0=t1[:, :], in1=B_b[:, :], op=OP.add)
        nc.vector.tensor_scalar(out=xln[:, k, :], in0=t2[:, :],
                                scalar1=g_sl[:, k:k + 1], scalar2=b_sl[:, k:k + 1],
                                op0=OP.mult, op1=OP.add)


def build_layers3(mmdt=mybir.dt.float32r, nl=3):
    """Program A: nl transformer layers on hT [1024, 256]."""
    nc = bacc.Bacc("TRN2", target_bir_lowering=False, num_devices=8)
    _register_consts(nc, [1e-5, 1e-9])

    hT_in = nc.dram_tensor("hT_in", [D, T], FP, kind="ExternalInput")
    wq_d = nc.dram_tensor("wq", [nl, D, D], FPR, kind="ExternalInput")
    wk_d = nc.dram_tensor("wk", [nl, D, D], FPR, kind="ExternalInput")
    wv_d = nc.dram_tensor("wv", [nl, D, D], FPR, kind="ExternalInput")
    wo_d = nc.dram_tensor("wo", [nl, D, D], FPR, kind="ExternalInput")
    ln1g_d = nc.dram_tensor("ln1g", [nl, D], FP, kind="ExternalInput")
    ln1b_d = nc.dram_tensor("ln1b", [nl, D], FP, kind="ExternalInput")
    ln2g_d = nc.dram_tensor("ln2g", [nl, D], FP, kind="ExternalInput")
    ln2b_d = nc.dram_tensor("ln2b", [nl, D], FP, kind="ExternalInput")
    w1_d = nc.dram_tensor("w1", [nl, D, DFF], FPR, kind="ExternalInput")
    b1_d = nc.dram_tensor("b1", [nl, DFF], FP, kind="ExternalInput")
    w2_d = nc.dram_tensor("w2", [nl, DFF, D], FPR, kind="ExternalInput")
    b2_d = nc.dram_tensor("b2", [nl, D], FP, kind="ExternalInput")
    C2_d = nc.dram_tensor("C2", [128, T], FP, kind="ExternalInput")
    S2_d = nc.dram_tensor("S2", [128, T], FP, kind="ExternalInput")
    ROT_d = nc.dram_tensor("ROT", [128, 128], FPR, kind="ExternalInput")

    hT_out = nc.dram_tensor("hT_out", [D, T], FP, kind="ExternalOutput")
    ent_out = nc.dram_tensor("ent", [nl, 128, 2 * H], FP, kind="ExternalOutput")

    with tile.TileContext(nc) as tc, ExitStack() as ctx:
        persist = ctx.enter_context(tc.tile_pool(name="persist", bufs=1))
        wpool = ctx.enter_context(tc.tile_pool(name="wts", bufs=4))
        actp = ctx.enter_context(tc.tile_pool(name="acts", bufs=1))
        prmp = ctx.enter_context(tc.tile_pool(name="prm", bufs=2))
        sqp = ctx.enter_context(tc.tile_pool(name="lnsq", bufs=3))
        bc = ctx.enter_context(tc.tile_pool(name="lnbc", bufs=2))
        small1 = ctx.enter_context(tc.tile_pool(name="small1", bufs=4))
        smallp = ctx.enter_context(tc.tile_pool(name="smallp", bufs=4))
        attp = ctx.enter_context(tc.tile_pool(name="attp", bufs=3))
        ropep = ctx.enter_context(tc.tile_pool(name="ropep", bufs=3))
        psacc = ctx.enter_context(tc.tile_pool(name="psacc", bufs=3, space="PSUM"))
        psaux = ctx.enter_context(tc.tile_pool(name="psaux", bufs=2, space="PSUM"))
        pssc = ctx.enter_context(tc.tile_pool(name="pssc", bufs=2, space="PSUM"))
        pso = ctx.enter_context(tc.tile_pool(name="pso", bufs=1, space="PSUM"))

        pools = {"psst": psacc, "small1": small1, "lnsq": sqp, "lnbc": bc}

        hT = persist.tile([128, KT, T], FP, tag="hT")
        nc.sync.dma_start(out=hT[:, :, :],
                          in_=hT_in.rearrange("(kt p) t -> p kt t", p=128))
        ones_f = persist.tile([128, 1], FP, tag="onesf")
        nc.gpsimd.memset(ones_f[:, :], 1.0)
        ones_col = persist.tile([128, 1], FPR, tag="ones")
        nc.vector.tensor_copy(ones_col[:, :], ones_f[:, :])
        ident = persist.tile([128, 128], FP, tag="ident")
        make_identity(nc, ident[:, :])
        C2 = persist.tile([128, T], FP, tag="C2")
        nc.sync.dma_start(out=C2[:, :], in_=C2_d[:, :])
        S2 = persist.tile([128, T], FP, tag="S2")
        nc.sync.dma_start(out=S2[:, :], in_=S2_d[:, :])
        ROT = persist.tile([128, 128], FPR, tag="ROT")
        nc.sync.dma_start(out=ROT[:, :], in_=ROT_d[:, :])

        for li in range(nl):
            ln1g = prmp.tile([128, KT], FP, tag="ln1g")
            ln1b = prmp.tile([128, KT], FP, tag="ln1b")
            ln2g = prmp.tile([128, KT], FP, tag="ln2g")
            ln2b = prmp.tile([128, KT], FP, tag="ln2b")
            nc.sync.dma_start(out=ln1g[:, :], in_=ln1g_d[li].rearrange("(kt p) -> p kt", p=128))
            nc.sync.dma_start(out=ln1b[:, :], in_=ln1b_d[li].rearrange("(kt p) -> p kt", p=128))
            nc.sync.dma_start(out=ln2g[:, :], in_=ln2g_d[li].rearrange("(kt p) -> p kt", p=128))
            nc.sync.dma_start(out=ln2b[:, :], in_=ln2b_d[li].rearrange("(kt p) -> p kt", p=128))
            b1t = prmp.tile([128, DFF // 128], FP, tag="b1t")
            nc.sync.dma_start(out=b1t[:, :], in_=b1_d[li].rearrange("(m p) -> p m", p=128))
            b2t = prmp.tile([128, KT], FP, tag="b2t")
            nc.sync.dma_start(out=b2t[:, :], in_=b2_d[li].rearrange("(m p) -> p m", p=128))

            # ---------- LN1 ----------
            xln = actp.tile([128, KT, T], FPR, tag="xln")
            layernorm(nc, pools, hT, ln1g, ln1b, xln, ones_col, mmdt)

            # ---------- QKV ----------
            qT = actp.tile([128, KT, T], FPR, tag="qT")
            kTt = actp.tile([128, KT, T], FPR, tag="kTt")
            vtok = actp.tile([128, 2, D], FPR, tag="vtok")

            for (wd, dst) in ((wq_d, qT), (wk_d, kTt)):
                for c in range(2):  # 512-col chunks
                    wc = wpool.tile([128, KT, 512], FPR, tag="wchunk")
                    wsrc = wd[li].rearrange("(kt p) o -> p kt o", p=128)
                    nc.sync.dma_start(out=wc[:, 0:4, :], in_=wsrc[:, 0:4, c * 512:(c + 1) * 512])
                    nc.sync.dma_start(out=wc[:, 4:8, :], in_=wsrc[:, 4:8, c * 512:(c + 1) * 512])
                    for jj in range(4):
                        j = c * 4 + jj
                        ps = psacc.tile([128, T], FP, tag="psacc")
                        for k in range(KT):
                            _mm(nc, ps[:, :], wc[:, k, jj * 128:(jj + 1) * 128],
                                xln[:, k, :], k == 0, k == KT - 1, mmdt)
                        qraw = ropep.tile([128, T], FPR, tag="qraw")
                        nc.vector.tensor_copy(qraw[:, :], ps[:, :])
                        psr = psaux.tile([128, T], FP, tag="psaux")
                        _mm(nc, psr[:, :], ROT[:, :], qraw[:, :], True, True, mmdt)
                        t1 = ropep.tile([128, T], FP, tag="ropet1")
                        nc.vector.tensor_tensor(out=t1[:, :], in0=qraw[:, :].bitcast(FP),
                                                in1=C2[:, :], op=OP.mult)
                        t2 = ropep.tile([128, T], FP, tag="ropet2")
                        nc.vector.tensor_tensor(out=t2[:, :], in0=psr[:, :],
                                                in1=S2[:, :], op=OP.mult)
                        nc.vector.tensor_tensor(out=dst[:, j, :], in0=t1[:, :],
                                                in1=t2[:, :], op=OP.add)

            for c in range(2):
                wc = wpool.tile([128, KT, 512], FPR, tag="wchunk")
                wsrc = wv_d[li].rearrange("(kt p) o -> p kt o", p=128)
                nc.sync.dma_start(out=wc[:, 0:4, :], in_=wsrc[:, 0:4, c * 512:(c + 1) * 512])
                nc.sync.dma_start(out=wc[:, 4:8, :], in_=wsrc[:, 4:8, c * 512:(c + 1) * 512])
                for i in range(2):
                    ps = psacc.tile([128, 512], FP, tag="psacc")
                    for k in range(KT):
                        _mm(nc, ps[:, :], xln[:, k, i * 128:(i + 1) * 128],
                            wc[:, k, :], k == 0, k == KT - 1, mmdt)
                    nc.scalar.copy(vtok[:, i, c * 512:(c + 1) * 512], ps[:, :])

            # ---------- attention ----------
            ocatT = actp.tile([128, KT, T], FPR, tag="ocatT")
            ent_sb = actp.tile([128, 2 * H], FP, tag="ent_sb")
            for b in range(2):
                for j in range(KT):
                    ps_o = pso.tile([128, 256], FP, tag="ps_o")
                    for hh in range(2):
                        h = 2 * j + hh
                        off = hh * 64
                        q_sl = qT[off:off + 64, j, b * 128:(b + 1) * 128]
                        k_sl = kTt[off:off + 64, j, b * 128:(b + 1) * 128]
                        ps_s = pssc.tile([128, 128], FP, tag="ps_s")
                        _mm(nc, ps_s[:, :], q_sl, k_sl, True, True, mmdt,
                            tile_position=(off, 0))
                        mx = smallp.tile([128, 1], FP, tag="mx")
                        nc.vector.tensor_reduce(out=mx[:, :], in_=ps_s[:, :],
                                                axis=AX.X, op=OP.max)
                        nbias = smallp.tile([128, 1], FP, tag="nbias")
                        nc.vector.tensor_scalar(out=nbias[:, :], in0=mx[:, :],
                                                scalar1=-INV_SCALE, scalar2=None,
                                                op0=OP.mult)
                        p_t = attp.tile([128, 128], FP, tag="p_t")
                        den = smallp.tile([128, 1], FP, tag="den")
                        nc.scalar.activation(p_t[:, :], ps_s[:, :], AF.Exp,
                                             bias=nbias[:, :], scale=INV_SCALE,
                                             accum_out=den[:, :])
                        rden = smallp.tile([128, 1], FP, tag="rden")
                        nc.vector.reciprocal(rden[:, :], den[:, :])
                        w_t = attp.tile([128, 128], FP, tag="w_t")
                        nc.vector.tensor_scalar(out=w_t[:, :], in0=p_t[:, :],
                                                scalar1=rden[:, :], scalar2=None,
                                                op0=OP.mult)
                        lnw = attp.tile([128, 128], FP, tag="lnw")
                        nc.scalar.activation(lnw[:, :], w_t[:, :], AF.Ln, bias=1e-9)
                        scr = attp.tile([128, 128], FP, tag="scr")
                        nc.vector.tensor_tensor(out=scr[:, :], in0=w_t[:, :],
                                                in1=lnw[:, :], op=OP.mult)
                        er = smallp.tile([128, 1], FP, tag="er")
                        nc.vector.tensor_reduce(out=er[:, :], in_=scr[:, :],
                                                axis=AX.X, op=OP.add)
                        nc.vector.tensor_scalar(
                            out=ent_sb[:, b * H + h:b * H + h + 1], in0=er[:, :],
                            scalar1=NLOG2E, scalar2=None, op0=OP.mult)
                        ps_t = psaux.tile([128, 128], FP, tag="psaux")
                        nc.tensor.transpose(ps_t[:, :], w_t[:, :], ident[:, :])
                        wT = attp.tile([128, 128], FPR, tag="wT")
                        nc.scalar.copy(wT[:, :], ps_t[:, :])
                        v_sl = vtok[:, b, h * HD:(h + 1) * HD]
                        _mm(nc, ps_o[0:64, hh * 128:hh * 128 + 128], v_sl, wT[:, :],
                            True, True, mmdt)
                    nc.vector.tensor_copy(ocatT[0:64, j, b * 128:(b + 1) * 128],
                                          ps_o[0:64, 0:128])
                    otmp = attp.tile([64, 128], FPR, tag="otmp")
                    nc.vector.tensor_copy(otmp[:, :], ps_o[0:64, 128:256])
                    nc.sync.dma_start(out=ocatT[64:128, j, b * 128:(b + 1) * 128],
                                      in_=otmp[:, :])
            nc.sync.dma_start(out=ent_out[li], in_=ent_sb[:, :])

            # ---------- wo + residual ----------
            for c in range(2):
                wc = wpool.tile([128, KT, 512], FPR, tag="wchunk")
                wsrc = wo_d[li].rearrange("(kt p) o -> p kt o", p=128)
                nc.sync.dma_start(out=wc[:, 0:4, :], in_=wsrc[:, 0:4, c * 512:(c + 1) * 512])
                nc.sync.dma_start(out=wc[:, 4:8, :], in_=wsrc[:, 4:8, c * 512:(c + 1) * 512])
                for jj in range(4):
                    m = c * 4 + jj
                    ps = psacc.tile([128, T], FP, tag="psacc")
                    for k in range(KT):
                        _mm(nc, ps[:, :], wc[:, k, jj * 128:(jj + 1) * 128],
                            ocatT[:, k, :], k == 0, k == KT - 1, mmdt)
                    nc.vector.tensor_tensor(out=hT[:, m, :], in0=ps[:, :],
                                            in1=hT[:, m, :], op=OP.add)

            # ---------- LN2 ----------
            xln2 = actp.tile([128, KT, T], FPR, tag="xln")
            layernorm(nc, pools, hT, ln2g, ln2b, xln2, ones_col, mmdt)

            # ---------- MLP w1 + gelu ----------
            geluT = actp.tile([128, DFF // 128, T], FPR, tag="geluT")
            for c in range(8):
                wc = wpool.tile([128, KT, 512], FPR, tag="wchunk")
                wsrc = w1_d[li].rearrange("(kt p) o -> p kt o", p=128)
                nc.sync.dma_start(out=wc[:, 0:4, :], in_=wsrc[:, 0:4, c * 512:(c + 1) * 512])
                nc.sync.dma_start(out=wc[:, 4:8, :], in_=wsrc[:, 4:8, c * 512:(c + 1) * 512])
                for jj in range(4):
                    m = c * 4 + jj
                    ps = psacc.tile([128, T], FP, tag="psacc")
                    for k in range(KT):
                        _mm(nc, ps[:, :], wc[:, k, jj * 128:(jj + 1) * 128],
                            xln2[:, k, :], k == 0, k == KT - 1, mmdt)
                    nc.scalar.activation(geluT[:, m, :], ps[:, :], AF.Gelu,
                                         bias=b1t[:, m:m + 1])

            # ---------- MLP w2 + residual + b2 ----------
            # per 4-ktile chunk: PSUM partial, then fused add into hT
            for c in range(8):
                wc = wpool.tile([128, 4, D], FPR, tag="wchunk")
                w2src = w2_d[li].rearrange("(kk p) o -> p kk o", p=128)
                nc.sync.dma_start(out=wc[:, 0:2, :], in_=w2src[:, c * 4:c * 4 + 2, :])
                nc.sync.dma_start(out=wc[:, 2:4, :], in_=w2src[:, c * 4 + 2:c * 4 + 4, :])
                for m in range(KT):
                    ps = psacc.tile([128, T], FP, tag="psacc")
                    for kk in range(4):
                        _mm(nc, ps[:, :], wc[:, kk, m * 128:(m + 1) * 128],
                            geluT[:, c * 4 + kk, :], kk == 0, kk == 3, mmdt)
                    if c < 7:
                        nc.vector.tensor_tensor(out=hT[:, m, :], in0=ps[:, :],
                                                in1=hT[:, m, :], op=OP.add)
                    else:
                        nc.vector.scalar_tensor_tensor(
                            out=hT[:, m, :], in0=ps[:, :], scalar=b2t[:, m:m + 1],
                            in1=hT[:, m, :], op0=OP.add, op1=OP.add)

        nc.sync.dma_start(out=hT_out.rearrange("(kt p) t -> p kt t", p=128),
                          in_=hT[:, :, :])
    nc.compile()
    return nc


def build_fcout(mmdt=mybir.dt.float32r):
    """Program B: logits[256, 32000] = h.T @ fo_w + fo_b."""
    nc = bacc.Bacc("TRN2", target_bir_lowering=False, num_devices=8)
    hT_in = nc.dram_tensor("hT_in", [D, T], FPR, kind="ExternalInput")
    fo_w = nc.dram_tensor("fo_w", [D, V], FPR, kind="ExternalInput")
    fo_b = nc.dram_tensor("fo_b", [1, V], FPR, kind="ExternalInput")
    logits = nc.dram_tensor("logits", [T, V], FP, kind="ExternalOutput")

    chunks = [(c * 512, 512) for c in range(62)] + [(62 * 512, V - 62 * 512)]

    with tile.TileContext(nc) as tc, ExitStack() as ctx:
        persist = ctx.enter_context(tc.tile_pool(name="persist", bufs=1))
        wpool = ctx.enter_context(tc.tile_pool(name="wts", bufs=4))
        bpool = ctx.enter_context(tc.tile_pool(name="fob", bufs=3))
        bbpool = ctx.enter_context(tc.tile_pool(name="fobb", bufs=3))
        opool = ctx.enter_context(tc.tile_pool(name="out", bufs=4))
        psacc = ctx.enter_context(tc.tile_pool(name="psacc", bufs=4, space="PSUM"))

        hT = persist.tile([128, KT, T], FPR, tag="hT")
        nc.sync.dma_start(out=hT[:, :, :],
                          in_=hT_in.rearrange("(kt p) t -> p kt t", p=128))
        for (o0, ow) in chunks:
            wc = wpool.tile([128, KT, 512], FPR, tag="fwc")
            fsrc = fo_w.rearrange("(kt p) o -> p kt o", p=128)
            nc.sync.dma_start(out=wc[:, 0:4, :ow], in_=fsrc[:, 0:4, o0:o0 + ow])
            nc.sync.dma_start(out=wc[:, 4:8, :ow], in_=fsrc[:, 4:8, o0:o0 + ow])
            fobb = bbpool.tile([128, 512], FPR, tag="fobb")
            nc.sync.dma_start(out=fobb[:, :ow],
                              in_=fo_b[:, o0:o0 + ow].to_broadcast((128, ow)))
            for i in range(2):
                ps = psacc.tile([128, 512], FP, tag="psfc")
                for k in range(KT):
                    _mm(nc, ps[:, :ow], hT[:, k, i * 128:(i + 1) * 128],
                        wc[:, k, :ow], k == 0, k == KT - 1, mmdt)
                ot = opool.tile([128, 512], FP, tag="ot")
                nc.vector.tensor_tensor(out=ot[:, :ow], in0=ps[:, :ow],
                                        in1=fobb[:, :ow].bitcast(FP), op=OP.add)
                nc.sync.dma_start(out=logits[i * 128:(i + 1) * 128, o0:o0 + ow],
                                  in_=ot[:, :ow])
    nc.compile()
    return nc


def host_consts():
    """C2, S2 [128, 256] and ROT [128, 128] fp32 (interleaved rope pairs).

    qT o-tile rows: [head0 d0..63, head1 d0..63], d pairs interleaved.
    C2[r, t] = cos[(r % 64) // 2, t % 128]; rot[2p] = -q[2p+1], rot[2p+1] = q[2p].
    """
    hd = HD
    inv = 1.0 / (10000.0 ** (np.arange(0, hd, 2)[: hd // 2].astype(np.float32) / hd))
    ang = np.outer(np.arange(128, dtype=np.float32), inv)  # [S=128, 32]
    cos = np.cos(ang).astype(np.float32)  # [128 pos, 32 pair]
    sin = np.sin(ang).astype(np.float32)
    C2 = np.zeros((128, T), np.float32)
    S2 = np.zeros((128, T), np.float32)
    for r in range(128):
        p = (r % 64) // 2
        for b in range(2):
            C2[r, b * 128:(b + 1) * 128] = cos[:, p]
            S2[r, b * 128:(b + 1) * 128] = sin[:, p]
    ROT = np.zeros((128, 128), np.float32)
    for p in range(64):
        ROT[2 * p + 1, 2 * p] = -1.0  # out[2p]   = -q[2p+1]
        ROT[2 * p, 2 * p + 1] = 1.0   # out[2p+1] = +q[2p]
    return C2, S2, ROT


# ======================================================================
import os
import numpy as np

from concourse.bass_utils import run_bass_kernel_spmd

NCORES = 8
B, S, D, V, L = 16, 128, 1024, 32000, 6
T = 256

_CACHE = {}
LAST_EXEC_NS = []


def _programs():
    import concourse.mybir as mybir
    key = "progs"
    if key not in _CACHE:
        _CACHE[key] = (build_layers3(mmdt=mybir.dt.float32r, nl=3),
                       build_fcout(mmdt=mybir.dt.float32r))
    return _CACHE[key]


def _trace_on():
    return bool(os.environ.get("KTRACE"))


def _install_shim():
    import sys, types
    if 'antenv.axon_hooks' not in sys.modules:
        sys.path.insert(0, '/root/.axon_site')
        from trn_agent_boot.trn_boot import _ntff_profile_via_ctypes
        hook = _ntff_profile_via_ctypes('/opt/axon/libaxon_pjrt.so')
        mod = types.ModuleType('antenv.axon_hooks')
        mod.get_axon_ntff_profile_hook = lambda: hook
        mod.set_axon_ntff_profile_hook = lambda h: None
        sys.modules['antenv.axon_hooks'] = mod


# ---------- fast path: persistent jit + device-resident weights ----------

def _runner(nc, tag):
    """Build (once) a jitted shard_map callable for `nc` over 8 cores."""
    key = ("runner", tag)
    if key in _CACHE:
        return _CACHE[key]
    import jax
    import concourse.mybir as mybir
    from concourse import bass2jax
    from jax.sharding import Mesh, PartitionSpec, NamedSharding
    from jax.experimental.shard_map import shard_map
    bass2jax.install_neuronx_cc_hook()

    part_name = (nc.partition_id_tensor.name if nc.partition_id_tensor
                 else None)
    in_names, out_names, out_avals = [], [], []
    for alloc in nc.m.functions[0].allocations:
        if not isinstance(alloc, mybir.MemoryLocationSet):
            continue
        name = alloc.memorylocations[0].name
        if alloc.kind == "ExternalInput":
            if name != part_name:
                in_names.append(name)
        elif alloc.kind == "ExternalOutput":
            out_names.append(name)
            out_avals.append(jax.core.ShapedArray(
                tuple(alloc.tensor_shape), mybir.dt.np(alloc.dtype)))
    bind_names = list(in_names) + list(out_names)
    if part_name is not None:
        bind_names.append(part_name)
    bind_names = tuple(bind_names)
    n_in = len(in_names)

    def _body(*args):
        operands = list(args)
        if part_name is not None:
            operands.append(bass2jax.partition_id_tensor())
        outs = bass2jax._bass_exec_p.bind(
            *operands, out_avals=tuple(out_avals), in_names=bind_names,
            out_names=tuple(out_names), lowering_input_output_aliases=(),
            sim_require_finite=True, sim_require_nnan=True, nc=nc)
        return tuple(outs)

    mesh = Mesh(np.asarray(jax.devices()[:NCORES]), ("core",))
    spec = PartitionSpec("core")
    nsh = NamedSharding(mesh, spec)
    n_out = len(out_names)
    fn = jax.jit(
        shard_map(_body, mesh=mesh, in_specs=(spec,) * (n_in + n_out),
                  out_specs=(spec,) * n_out, check_rep=False),
        donate_argnums=tuple(range(n_in, n_in + n_out)), keep_unused=True)
    r = (fn, in_names, out_names, out_avals, nsh)
    _CACHE[key] = r
    return r


def _stage(name, arr, nsh, replicate=True):
    """device_put a per-core-replicated (or already stacked) array, cached."""
    import jax
    key = ("dev", name)
    if key not in _CACHE:
        big = np.concatenate([arr] * NCORES, axis=0) if replicate else arr
        _CACHE[key] = jax.device_put(big, nsh)
    return _CACHE[key]


def _zeros(shape, dtype, nsh):
    import jax, jax.numpy as jnp
    key = ("zfn", shape, str(dtype))
    if key not in _CACHE:
        _CACHE[key] = jax.jit(lambda: jnp.zeros(shape, dtype),
                              out_shardings=nsh)
    return _CACHE[key]()


def _run_fast(nc, tag, dyn_inputs, static_inputs):
    """dyn_inputs: name -> stacked np/jax array [8*d0, ...] (per-call);
    static_inputs: name -> (per-core np array, cache_key) staged once."""
    import jax
    fn, in_names, out_names, out_avals, nsh = _runner(nc, tag)
    args = []
    for name in in_names:
        if name in dyn_inputs:
            v = dyn_inputs[name]
            if isinstance(v, np.ndarray):
                v = jax.device_put(v, nsh)
            args.append(v)
        else:
            arr, ck = static_inputs[name]
            args.append(_stage(ck, arr, nsh))
    for av in out_avals:
        args.append(_zeros((NCORES * av.shape[0],) + av.shape[1:], av.dtype, nsh))
    outs = fn(*args)
    return dict(zip(out_names, outs))


# ---------- traced path (timing) ----------

def _run_traced(nc, in_maps, label):
    _install_shim()
    res = run_bass_kernel_spmd(nc, in_maps, core_ids=list(range(NCORES)),
                               trace=True)
    if res.exec_time_ns is not None:
        LAST_EXEC_NS.append((label, res.exec_time_ns))
    return res.results


def kernel(x, emb, wq, wk, wv, wo, ln1_g, ln1_b, w1, b1, w2, b2, ln2_g, ln2_b,
           fo_w, fo_b):
    del LAST_EXEC_NS[:]
    x = np.asarray(x)
    f32 = lambda a: np.ascontiguousarray(np.asarray(a, np.float32))
    emb = f32(emb)
    wq, wk, wv, wo = f32(wq), f32(wk), f32(wv), f32(wo)
    w1, w2, b1, b2 = f32(w1), f32(w2), f32(b1), f32(b2)
    ln1_g, ln1_b, ln2_g, ln2_b = f32(ln1_g), f32(ln1_b), f32(ln2_g), f32(ln2_b)
    fo_w, fo_b = f32(fo_w), f32(fo_b)

    ncA, ncB = _programs()
    C2, S2, ROT = host_consts()

    h0 = emb[x.astype(np.int64)]  # [16, 128, 1024]
    hT0 = np.concatenate(
        [np.ascontiguousarray(h0[2 * c:2 * c + 2].reshape(T, D).T)
         for c in range(NCORES)], axis=0)  # [8*1024, 256]

    wsig = float(np.float64(wq[0, 0, 0]))  # cache buster across weight sets

    def a_static(lo):
        sl = slice(lo, lo + 3)
        names = {'wq': wq[sl], 'wk': wk[sl], 'wv': wv[sl], 'wo': wo[sl],
                 'ln1g': ln1_g[sl], 'ln1b': ln1_b[sl],
                 'ln2g': ln2_g[sl], 'ln2b': ln2_b[sl],
                 'w1': w1[sl], 'b1': b1[sl], 'w2': w2[sl], 'b2': b2[sl],
                 'C2': C2, 'S2': S2, 'ROT': ROT}
        return {k: (v, (k, lo, wsig)) for k, v in names.items()}

    use_traced = _trace_on()

    def runA(hT_stacked, lo, label):
        if use_traced:
            # split stacked into per-core maps
            hTs = np.asarray(hT_stacked).reshape(NCORES, D, T)
            com = {k: v for k, (v, _) in a_static(lo).items()}
            maps = [{**com, 'hT_in': hTs[c]} for c in range(NCORES)]
            r = _run_traced(ncA, maps, label)
            hT_next = np.concatenate([r[c]['hT_out'] for c in range(NCORES)])
            ents = np.stack([r[c]['ent'] for c in range(NCORES)])
            return hT_next, ents
        out = _run_fast(ncA, "A", {'hT_in': hT_stacked}, a_static(lo))
        ents = np.asarray(out['ent']).reshape(NCORES, 3, 128, 2 * 16)
        return out['hT_out'], ents

    hT1, ent1 = runA(hT0, 0, "A1")
    e = ent1.reshape(NCORES, 3, S, 2, 16).transpose(1, 0, 3, 2, 4)
    e = e.reshape(3, B * S, 16).astype(np.float32)
    g = np.mean([np.var(e[l], axis=-1, ddof=1).mean() for l in range(3)])

    if g < 0.6:
        hT2, _ = runA(hT1, 0, "A2")
    else:
        hT2 = hT1
    hT3, _ = runA(hT2, 3, "A3")

    fob2 = fo_b.reshape(1, V)
    if use_traced:
        hTs = np.asarray(hT3).reshape(NCORES, D, T)
        maps = [{'hT_in': hTs[c], 'fo_w': fo_w, 'fo_b': fob2}
                for c in range(NCORES)]
        rb = _run_traced(ncB, maps, "B")
        logits = np.stack([rb[c]['logits'] for c in range(NCORES)])
    else:
        outb = _run_fast(ncB, "B", {'hT_in': hT3},
                         {'fo_w': (fo_w, ('fo_w', wsig)),
                          'fo_b': (fob2, ('fo_b', wsig))})
        logits = np.asarray(outb['logits']).reshape(NCORES, T, V)

    out = np.empty((B, S, V), np.float32)
    for c in range(NCORES):
        out[2 * c:2 * c + 2] = logits[c].reshape(2, S, V)
    return out



# revision 18
# speedup vs baseline: 1.5738x; 1.5738x over previous
"""Self-contained Trainium2 Bass kernel for nn_AdvancedGenuineTransformer_35485019799944.

kernel(**inputs) -> np.ndarray  (full [16,128,32000] logits)

Design (v2):
- Data-parallel over batch: each of 8 cores handles 2 batches (T=256 tokens).
- All weights streamed as bf16 in host-pre-packed, DMA-contiguous layouts.
- LayerNorm gains/biases folded into the adjacent weight matrices on host
  (exact): w' = diag(g) @ w, bias row s_b = b @ w added via K=1 matmuls.
- Entropy computed from raw scores (H = ln(den) - inv*r2/den), batched Ln
  once per layer -> no per-head activation-table swaps.
- fc_out vocab-sharded: each core computes all 2048 tokens x 4000 vocab.
"""


import sys

sys.path.insert(0, "/opt/trn_rl_repo")

from contextlib import ExitStack

import numpy as np

import concourse.bacc as bacc
import concourse.mybir as mybir
import concourse.tile as tile
from concourse.masks import make_identity

FP = mybir.dt.float32
FPR = mybir.dt.float32r
BF = mybir.dt.bfloat16
AF = mybir.ActivationFunctionType
OP = mybir.AluOpType
AX = mybir.AxisListType

D = 1024
T = 256          # tokens per core (2 batches x 128)
KT = 8           # d tiles
H = 16
HD = 64
DFF = 4096
V = 32000
VS = V // 8      # vocab slice per core
TFULL = 2048     # all tokens (for fc_out)
INV_SCALE = 0.125          # 1/sqrt(64)
LOG2E = 1.4426950408889634


def _register_consts(nc, values):
    for v in values:
        t = nc.alloc_sbuf_tensor(f"const-float32-{v}", [128, 1], FP)
        nc.gpsimd.memset(t.ap(), v)
        nc.const_aps.aps[(FP, v)] = t.ap()
    nc.all_engine_barrier()


def _mm(nc, ps, lhsT, rhs, start, stop):
    nc.tensor.matmul(ps, lhsT, rhs, start=start, stop=stop)


def build_layers(nl=3):
    """Program A: nl transformer layers on hT [1024, 256] (fp32 residual)."""
    nc = bacc.Bacc("TRN2", target_bir_lowering=False, num_devices=8)
    _register_consts(nc, [1e-5])

    hT_in = nc.dram_tensor("hT_in", [D, T], FPR, kind="ExternalInput")
    wq_d = nc.dram_tensor("wq", [nl, 128, KT, D], BF, kind="ExternalInput")
    wk_d = nc.dram_tensor("wk", [nl, 128, KT, D], BF, kind="ExternalInput")
    wv_d = nc.dram_tensor("wv", [nl, 128, KT, D], BF, kind="ExternalInput")
    wo_d = nc.dram_tensor("wo", [nl, 128, KT, D], BF, kind="ExternalInput")
    w1_d = nc.dram_tensor("w1", [nl, 4, 128, KT, D], BF, kind="ExternalInput")
    w2_d = nc.dram_tensor("w2", [nl, 4, 128, KT, D], BF, kind="ExternalInput")
    pb_d = nc.dram_tensor("pb", [nl, 128, 40], FP, kind="ExternalInput")
    sb_d = nc.dram_tensor("sb", [nl, 1, 3 * D], BF, kind="ExternalInput")
    C2_d = nc.dram_tensor("C2", [128, T], FP, kind="ExternalInput")
    S2_d = nc.dram_tensor("S2", [128, T], FP, kind="ExternalInput")
    ROT_d = nc.dram_tensor("ROT", [128, 128], FPR, kind="ExternalInput")

    hT_out = nc.dram_tensor("hT_out", [D, T], FPR, kind="ExternalOutput")
    ent_out = nc.dram_tensor("ent", [nl, 128, 2 * H], FP, kind="ExternalOutput")

    with tile.TileContext(nc) as tc, ExitStack() as ctx:
        persist = ctx.enter_context(tc.tile_pool(name="persist", bufs=1))
        prmp = ctx.enter_context(tc.tile_pool(name="prm", bufs=2))
        wpool = ctx.enter_context(tc.tile_pool(name="wts", bufs=6))
        actp = ctx.enter_context(tc.tile_pool(name="acts", bufs=1))
        ropep = ctx.enter_context(tc.tile_pool(name="rope", bufs=3))
        sqp = ctx.enter_context(tc.tile_pool(name="sq", bufs=8))
        attp = ctx.enter_context(tc.tile_pool(name="attp", bufs=3))
        entp = ctx.enter_context(tc.tile_pool(name="entp", bufs=2))
        bcp = ctx.enter_context(tc.tile_pool(name="bcp", bufs=2))
        sm1 = ctx.enter_context(tc.tile_pool(name="sm1", bufs=4))
        # PSUM: slots are bank-granular (2KB/partition); total tags*bufs <= 8
        psm = ctx.enter_context(tc.tile_pool(name="psm", bufs=4, space="PSUM"))
        paux = ctx.enter_context(tc.tile_pool(name="paux", bufs=2, space="PSUM"))
        pst2 = ctx.enter_context(tc.tile_pool(name="pst2", bufs=1, space="PSUM"))
        pso = ctx.enter_context(tc.tile_pool(name="pso", bufs=1, space="PSUM"))

        hT = persist.tile([128, KT, T], FPR, tag="hT")
        nc.sync.dma_start(out=hT[:, :, :],
                          in_=hT_in.rearrange("(kt p) t -> p kt t", p=128))
        ones_f = persist.tile([128, 1], FP, tag="onesf")
        nc.gpsimd.memset(ones_f[:, :], 1.0)
        ones_col = persist.tile([128, 1], FPR, tag="onesc")
        nc.vector.tensor_copy(ones_col[:, :], ones_f[:, :])
        ones_row = persist.tile([1, T], BF, tag="onesr")
        nc.gpsimd.memset(ones_row[:, :], 1.0)
        identb = persist.tile([128, 128], BF, tag="identb")
        make_identity(nc, identb[:, :])
        C2 = persist.tile([128, T], FP, tag="C2")
        nc.sync.dma_start(out=C2[:, :], in_=C2_d[:, :])
        S2 = persist.tile([128, T], FP, tag="S2")
        nc.sync.dma_start(out=S2[:, :], in_=S2_d[:, :])
        ROT = persist.tile([128, 128], FPR, tag="ROT")
        nc.sync.dma_start(out=ROT[:, :], in_=ROT_d[:, :])

        def layernorm(xln, gen):
            """xln (bf16) = (hT - mean) * rsqrt(var + eps); stats over d."""
            ps_sum = psm.tile([1, T], FP, tag="psm")
            ps_sq = psm.tile([1, T], FP, tag="psm")
            sqs = []
            for k in range(KT):
                sq = sqp.tile([128, T], FPR, tag="sq")
                nc.scalar.activation(sq[:, :], hT[:, k, :], AF.Square)
                sqs.append(sq)
            for k in range(KT):
                _mm(nc, ps_sum[:, :], ones_col[:, :1],
                    hT[:, k, :], k == 0, k == KT - 1)
            for k in range(KT):
                _mm(nc, ps_sq[:, :], ones_col[:, :1],
                    sqs[k][:, :], k == 0, k == KT - 1)
            mean = sm1.tile([1, T], FP, tag="sm1")
            nc.scalar.mul(mean[:, :], ps_sum[:, :], 1.0 / D)
            msq = sm1.tile([1, T], FP, tag="sm1")
            nc.scalar.mul(msq[:, :], ps_sq[:, :], 1.0 / D)
            m2 = sm1.tile([1, T], FP, tag="sm1")
            nc.vector.tensor_tensor(out=m2[:, :], in0=mean[:, :], in1=mean[:, :],
                                    op=OP.mult)
            var = sm1.tile([1, T], FP, tag="sm1")
            nc.vector.tensor_tensor(out=var[:, :], in0=msq[:, :], in1=m2[:, :],
                                    op=OP.subtract)
            std = sm1.tile([1, T], FP, tag="sm1")
            nc.scalar.activation(std[:, :], var[:, :], AF.Sqrt, bias=1e-5)
            rs = sm1.tile([1, T], FP, tag="sm1")
            nc.vector.reciprocal(rs[:, :], std[:, :])
            nm = sm1.tile([1, T], FP, tag="sm1")
            nc.vector.scalar_tensor_tensor(out=nm[:, :], in0=mean[:, :],
                                           scalar=-1.0, in1=rs[:, :],
                                           op0=OP.mult, op1=OP.mult)
            A_b = bcp.tile([128, T], FP, tag=f"A{gen}")
            B_b = bcp.tile([128, T], FP, tag=f"B{gen}")
            nc.gpsimd.partition_broadcast(A_b[:, :], rs[:, :])
            nc.gpsimd.partition_broadcast(B_b[:, :], nm[:, :])
            for k in range(KT):
                t1 = ropep.tile([128, T], FP, tag="lnt")
                nc.vector.tensor_tensor(out=t1[:, :], in0=hT[:, k, :],
                                        in1=A_b[:, :], op=OP.mult)
                nc.vector.tensor_tensor(out=xln[:, k, :], in0=t1[:, :],
                                        in1=B_b[:, :], op=OP.add)

        for li in range(nl):
            pb = prmp.tile([128, 40], FP, tag="pb")
            nc.scalar.dma_start(out=pb[:, :], in_=pb_d[li])
            sb = prmp.tile([1, 3 * D], BF, tag="sb")
            nc.scalar.dma_start(out=sb[:, :], in_=sb_d[li])

            # ---------- LN1 ----------
            xln = actp.tile([128, KT, T], BF, tag="xln", bufs=2)
            layernorm(xln, 0)

            # ---------- QKV ----------
            wcq = wpool.tile([128, KT, D], BF, tag="w")
            nc.sync.dma_start(out=wcq[:, :, :], in_=wq_d[li])
            wck = wpool.tile([128, KT, D], BF, tag="w")
            nc.sync.dma_start(out=wck[:, :, :], in_=wk_d[li])
            wcv = wpool.tile([128, KT, D], BF, tag="w")
            nc.sync.dma_start(out=wcv[:, :, :], in_=wv_d[li])

            qT = actp.tile([128, KT, T], BF, tag="qT")
            kTt = actp.tile([128, KT, T], BF, tag="kTt")
            vtok = actp.tile([128, 2, D], BF, tag="vtok")

            for (wc, dst, sboff) in ((wcq, qT, 0), (wck, kTt, D)):
                for j in range(KT):
                    ps = psm.tile([128, T], FP, tag="psm")
                    for k in range(KT):
                        _mm(nc, ps[:, :], wc[:, k, j * 128:(j + 1) * 128],
                            xln[:, k, :], k == 0, False)
                    _mm(nc, ps[:, :], sb[0:1, sboff + j * 128:sboff + (j + 1) * 128],
                        ones_row[0:1, :], False, True)
                    qraw = ropep.tile([128, T], FPR, tag="qraw")
                    nc.scalar.copy(qraw[:, :], ps[:, :])
                    psr = paux.tile([128, T], FP, tag="paux")
                    _mm(nc, psr[:, :], ROT[:, :], qraw[:, :], True, True)
                    t1 = ropep.tile([128, T], FP, tag="t1")
                    nc.gpsimd.tensor_tensor(out=t1[:, :], in0=qraw[:, :],
                                            in1=C2[:, :], op=OP.mult)
                    t2 = ropep.tile([128, T], FP, tag="t2")
                    nc.vector.tensor_tensor(out=t2[:, :], in0=psr[:, :],
                                            in1=S2[:, :], op=OP.mult)
                    nc.vector.tensor_tensor(out=dst[:, j, :], in0=t1[:, :],
                                            in1=t2[:, :], op=OP.add)

            for c in range(2):
                for i in range(2):
                    ps = psm.tile([128, 512], FP, tag="psm")
                    for k in range(KT):
                        _mm(nc, ps[:, :], xln[:, k, i * 128:(i + 1) * 128],
                            wcv[:, k, c * 512:(c + 1) * 512], k == 0, False)
                    _mm(nc, ps[:, :], ones_row[0:1, 0:128],
                        sb[0:1, 2 * D + c * 512:2 * D + (c + 1) * 512],
                        False, True)
                    nc.scalar.copy(vtok[:, i, c * 512:(c + 1) * 512], ps[:, :])

            # prefetch wo while attention runs
            wco = wpool.tile([128, KT, D], BF, tag="w")
            nc.sync.dma_start(out=wco[:, :, :], in_=wo_d[li])

            # ---------- attention ----------
            ocatT = actp.tile([128, KT, T], BF, tag="ocatT")
            den_all = entp.tile([128, 2 * H], FP, tag="den")
            r2_all = entp.tile([128, 2 * H], FP, tag="r2")
            rden_all = entp.tile([128, 2 * H], FP, tag="rden")
            for b in range(2):
                for j in range(KT):
                    ps_s = paux.tile([128, 256], FP, tag="paux")
                    ps_o = pso.tile([128, 128], FP, tag="pso")
                    ps_t = pst2.tile([128, 256], BF, tag="pst2")
                    for hh in range(2):
                        h = 2 * j + hh
                        off = hh * 64
                        col = b * H + h
                        q_sl = qT[off:off + 64, j, b * 128:(b + 1) * 128]
                        k_sl = kTt[off:off + 64, j, b * 128:(b + 1) * 128]
                        s_sl = ps_s[:, hh * 128:(hh + 1) * 128]
                        _mm(nc, s_sl, q_sl, k_sl, True, True)
                        p_t = attp.tile([128, 128], FP, tag="p_t")
                        nc.scalar.activation(p_t[:, :], s_sl, AF.Exp,
                                             bias=0.0, scale=INV_SCALE,
                                             accum_out=den_all[:, col:col + 1])
                        scr = attp.tile([128, 128], FP, tag="scr")
                        nc.vector.scalar_tensor_tensor(
                            out=scr[:, :], in0=s_sl, scalar=INV_SCALE,
                            in1=p_t[:, :], op0=OP.mult, op1=OP.mult)
                        nc.vector.tensor_reduce(
                            out=r2_all[:, col:col + 1], in_=scr[:, :],
                            axis=AX.X, op=OP.add)
                        nc.vector.reciprocal(rden_all[:, col:col + 1],
                                             den_all[:, col:col + 1])
                        w_t = attp.tile([128, 128], BF, tag="w_t")
                        nc.vector.tensor_scalar(
                            out=w_t[:, :], in0=p_t[:, :],
                            scalar1=rden_all[:, col:col + 1], scalar2=None,
                            op0=OP.mult)
                        nc.tensor.transpose(ps_t[:, hh * 128:(hh + 1) * 128],
                                            w_t[:, :], identb[:, :])
                        wT = attp.tile([128, 128], BF, tag="wT")
                        nc.scalar.copy(wT[:, :], ps_t[:, hh * 128:(hh + 1) * 128])
                        v_sl = vtok[:, b, h * HD:(h + 1) * HD]
                        _mm(nc, ps_o[off:off + 64, :], v_sl, wT[:, :],
                            True, True)
                    nc.scalar.copy(ocatT[:, j, b * 128:(b + 1) * 128],
                                   ps_o[:, :])

            # per-layer entropy: H_bits = LOG2E*(ln(den) - INV*r2/den)
            lnden = entp.tile([128, 2 * H], FP, tag="lnden")
            nc.scalar.activation(lnden[:, :], den_all[:, :], AF.Ln, bias=0.0)
            tq = entp.tile([128, 2 * H], FP, tag="tq")
            nc.vector.tensor_tensor(out=tq[:, :], in0=r2_all[:, :],
                                    in1=rden_all[:, :], op=OP.mult)
            uq = entp.tile([128, 2 * H], FP, tag="uq")
            nc.vector.scalar_tensor_tensor(out=uq[:, :], in0=tq[:, :],
                                           scalar=-1.0, in1=lnden[:, :],
                                           op0=OP.mult, op1=OP.add)
            ent_sb = entp.tile([128, 2 * H], FP, tag="ent")
            nc.vector.tensor_scalar(out=ent_sb[:, :], in0=uq[:, :],
                                    scalar1=LOG2E, scalar2=None, op0=OP.mult)
            nc.scalar.dma_start(out=ent_out[li], in_=ent_sb[:, :])

            # ---------- wo + residual ----------
            for m in range(KT):
                ps = psm.tile([128, T], FP, tag="psm")
                for k in range(KT):
                    _mm(nc, ps[:, :], wco[:, k, m * 128:(m + 1) * 128],
                        ocatT[:, k, :], k == 0, k == KT - 1)
                nc.vector.tensor_tensor(out=hT[:, m, :], in0=ps[:, :],
                                        in1=hT[:, m, :], op=OP.add)

            # ---------- LN2 ----------
            xln2 = actp.tile([128, KT, T], BF, tag="xln", bufs=2)
            layernorm(xln2, 1)

            # ---------- MLP w1 + gelu (b1' via gelu bias) ----------
            geluT = actp.tile([128, DFF // 128, T], BF, tag="geluT")
            for c in range(4):
                wc1 = wpool.tile([128, KT, D], BF, tag="w")
                nc.sync.dma_start(out=wc1[:, :, :], in_=w1_d[li, c])
                for jj in range(KT):
                    m = c * KT + jj
                    ps = psm.tile([128, T], FP, tag="psm")
                    for k in range(KT):
                        _mm(nc, ps[:, :], wc1[:, k, jj * 128:(jj + 1) * 128],
                            xln2[:, k, :], k == 0, k == KT - 1)
                    nc.scalar.activation(geluT[:, m, :], ps[:, :], AF.Gelu,
                                         bias=pb[:, m:m + 1])

            # ---------- MLP w2 + b2 + residual ----------
            wc2s = []
            for c in range(4):
                wc2 = wpool.tile([128, KT, D], BF, tag="w")
                nc.sync.dma_start(out=wc2[:, :, :], in_=w2_d[li, c])
                wc2s.append(wc2)
            for m in range(KT):
                ps = psm.tile([128, T], FP, tag="psm")
                for c in range(4):
                    for kk in range(KT):
                        _mm(nc, ps[:, :], wc2s[c][:, kk, m * 128:(m + 1) * 128],
                            geluT[:, c * KT + kk, :],
                            c == 0 and kk == 0, c == 3 and kk == KT - 1)
                nc.vector.scalar_tensor_tensor(
                    out=hT[:, m, :], in0=ps[:, :], scalar=pb[:, 32 + m:33 + m],
                    in1=hT[:, m, :], op0=OP.add, op1=OP.add)

        nc.sync.dma_start(out=hT_out.rearrange("(kt p) t -> p kt t", p=128),
                          in_=hT[:, :, :])
    nc.compile()
    return nc


def build_fcout():
    """Program B: vocab-sharded logits[2048, 4000] = h.T @ fo_w_slice + fo_b."""
    nc = bacc.Bacc("TRN2", target_bir_lowering=False, num_devices=8)
    NCH = 8
    CW = VS // NCH  # 500
    hT_in = nc.dram_tensor("hT_in", [128, KT, TFULL], BF, kind="ExternalInput")
    fw_d = nc.dram_tensor("fo_w", [NCH, 128, KT, CW], BF, kind="ExternalInput")
    fob_d = nc.dram_tensor("fo_b", [1, VS], BF, kind="ExternalInput")
    logits = nc.dram_tensor("logits", [TFULL, VS], FP, kind="ExternalOutput")

    with tile.TileContext(nc) as tc, ExitStack() as ctx:
        persist = ctx.enter_context(tc.tile_pool(name="persist", bufs=1))
        opool = ctx.enter_context(tc.tile_pool(name="out", bufs=3))
        pspool = ctx.enter_context(tc.tile_pool(name="ps", bufs=6, space="PSUM"))

        hTt = persist.tile([128, KT, TFULL], BF, tag="hTt")
        nc.sync.dma_start(out=hTt[:, :, :], in_=hT_in[:, :, :])
        ones_row = persist.tile([1, 128], BF, tag="onesr")
        nc.gpsimd.memset(ones_row[:, :], 1.0)
        fob = persist.tile([1, VS], BF, tag="fob")
        nc.scalar.dma_start(out=fob[:, :], in_=fob_d[:, :])
        wcs = []
        for c in range(NCH):
            wc = persist.tile([128, KT, CW], BF, tag=f"fw{c}")
            nc.sync.dma_start(out=wc[:, :, :], in_=fw_d[c])
            wcs.append(wc)

        for i in range(TFULL // 128):
            obuf = opool.tile([128, VS], FP, tag="ob")
            for c in range(NCH):
                ps = pspool.tile([128, CW], FP, tag="ps")
                for k in range(KT):
                    _mm(nc, ps[:, :], hTt[:, k, i * 128:(i + 1) * 128],
                        wcs[c][:, k, :], k == 0, False)
                _mm(nc, ps[:, :], ones_row[0:1, :],
                    fob[0:1, c * CW:(c + 1) * CW], False, True)
                if c % 2 == 0:
                    nc.scalar.copy(obuf[:, c * CW:(c + 1) * CW], ps[:, :])
                else:
                    nc.vector.tensor_copy(obuf[:, c * CW:(c + 1) * CW], ps[:, :])
            nc.sync.dma_start(out=logits[i * 128:(i + 1) * 128, :],
                              in_=obuf[:, :])
    nc.compile()
    return nc


def host_consts():
    """C2, S2 [128, 256] and ROT [128, 128] fp32 (interleaved rope pairs).

    qT o-tile rows: [head0 d0..63, head1 d0..63], d pairs interleaved.
    C2[r, t] = cos[(r % 64) // 2, t % 128]; rot[2p] = -q[2p+1], rot[2p+1] = q[2p].
    """
    hd = HD
    inv = 1.0 / (10000.0 ** (np.arange(0, hd, 2)[: hd // 2].astype(np.float32) / hd))
    ang = np.outer(np.arange(128, dtype=np.float32), inv)  # [S=128, 32]
    cos = np.cos(ang).astype(np.float32)  # [128 pos, 32 pair]
    sin = np.sin(ang).astype(np.float32)
    C2 = np.zeros((128, T), np.float32)
    S2 = np.zeros((128, T), np.float32)
    for r in range(128):
        p = (r % 64) // 2
        for b in range(2):
            C2[r, b * 128:(b + 1) * 128] = cos[:, p]
            S2[r, b * 128:(b + 1) * 128] = sin[:, p]
    ROT = np.zeros((128, 128), np.float32)
    for p in range(64):
        ROT[2 * p + 1, 2 * p] = -1.0  # out[2p]   = -q[2p+1]
        ROT[2 * p, 2 * p + 1] = 1.0   # out[2p+1] = +q[2p]
    return C2, S2, ROT


# ======================================================================
import os
import numpy as np
import ml_dtypes

BF_NP = ml_dtypes.bfloat16

from concourse.bass_utils import run_bass_kernel_spmd

NCORES = 8
B, S = 16, 128
L = 6

_CACHE = {}
LAST_EXEC_NS = []


def _programs():
    key = "progs"
    if key not in _CACHE:
        _CACHE[key] = (build_layers(nl=3), build_fcout())
    return _CACHE[key]


def _trace_on():
    return bool(os.environ.get("KTRACE"))


def _install_shim():
    import sys, types
    if 'antenv.axon_hooks' not in sys.modules:
        sys.path.insert(0, '/root/.axon_site')
        from trn_agent_boot.trn_boot import _ntff_profile_via_ctypes
        hook = _ntff_profile_via_ctypes('/opt/axon/libaxon_pjrt.so')
        mod = types.ModuleType('antenv.axon_hooks')
        mod.get_axon_ntff_profile_hook = lambda: hook
        mod.set_axon_ntff_profile_hook = lambda h: None
        sys.modules['antenv.axon_hooks'] = mod


# ---------- host weight packing (exact LN folding + bf16 + DMA layouts) ----

def _pack_qkvo(w, g):
    """[3,D,D] fp32, g [3,D] -> [3,128,KT,D] bf16 with rows scaled by g."""
    nl = w.shape[0]
    out = np.empty((nl, 128, KT, D), BF_NP)
    for i in range(nl):
        wg = w[i] * g[i][:, None]
        out[i] = wg.reshape(KT, 128, D).transpose(1, 0, 2).astype(BF_NP)
    return out


def _pack_w1(w1, g2):
    nl = w1.shape[0]
    out = np.empty((nl, 4, 128, KT, D), BF_NP)
    for i in range(nl):
        wg = w1[i] * g2[i][:, None]          # [D, 4096]
        out[i] = wg.reshape(KT, 128, 4, D).transpose(2, 1, 0, 3).astype(BF_NP)
    return out


def _pack_w2(w2):
    nl = w2.shape[0]
    out = np.empty((nl, 4, 128, KT, D), BF_NP)
    for i in range(nl):
        out[i] = w2[i].reshape(4, KT, 128, D).transpose(0, 2, 1, 3).astype(BF_NP)
    return out


def _pack_pb(b1, ln2_b, w1, b2):
    """pb [nl,128,40]: cols 0..31 = b1' = b1 + ln2_b @ w1 (per-ffn-dim),
    cols 32..39 = b2 (per-d)."""
    nl = b1.shape[0]
    out = np.zeros((nl, 128, 40), np.float32)
    for i in range(nl):
        b1p = b1[i] + ln2_b[i] @ w1[i]       # [4096]
        out[i, :, 0:32] = b1p.reshape(32, 128).T
        out[i, :, 32:40] = b2[i].reshape(KT, 128).T
    return out


def _pack_sb(ln1_b, wq, wk, wv):
    nl = wq.shape[0]
    out = np.zeros((nl, 1, 3 * D), BF_NP)
    for i in range(nl):
        out[i, 0, 0:D] = (ln1_b[i] @ wq[i]).astype(BF_NP)
        out[i, 0, D:2 * D] = (ln1_b[i] @ wk[i]).astype(BF_NP)
        out[i, 0, 2 * D:3 * D] = (ln1_b[i] @ wv[i]).astype(BF_NP)
    return out


def _pack_fo(fo_w):
    """[D, V] -> per-core [8, 128, KT, 500] bf16 slices, stacked [8*8,...]."""
    CW = VS // 8
    out = np.empty((NCORES, 8, 128, KT, CW), BF_NP)
    for c in range(NCORES):
        sl = fo_w[:, c * VS:(c + 1) * VS]      # [D, 4000]
        out[c] = sl.reshape(KT, 128, 8, CW).transpose(2, 1, 0, 3).astype(BF_NP)
    return out.reshape(NCORES * 8, 128, KT, CW)


def _pack_hT_full(hT3):
    """stacked [8*D, T] fp32 -> [128, KT, 2048] bf16 (replicated per core)."""
    hs = np.asarray(hT3).reshape(NCORES, D, T)
    Hfull = np.concatenate([hs[c] for c in range(NCORES)], axis=1)  # [D, 2048]
    return Hfull.reshape(KT, 128, TFULL).transpose(1, 0, 2).astype(BF_NP)


# ---------- fast path: persistent jit + device-resident weights ----------

def _runner(nc, tag):
    """Build (once) a jitted shard_map callable for `nc` over 8 cores."""
    key = ("runner", tag)
    if key in _CACHE:
        return _CACHE[key]
    import jax
    from concourse import bass2jax
    from jax.sharding import Mesh, PartitionSpec, NamedSharding
    from jax.experimental.shard_map import shard_map
    bass2jax.install_neuronx_cc_hook()

    part_name = (nc.partition_id_tensor.name if nc.partition_id_tensor
                 else None)
    in_names, out_names, out_avals = [], [], []
    for alloc in nc.m.functions[0].allocations:
        if not isinstance(alloc, mybir.MemoryLocationSet):
            continue
        name = alloc.memorylocations[0].name
        if alloc.kind == "ExternalInput":
            if name != part_name:
                in_names.append(name)
        elif alloc.kind == "ExternalOutput":
            out_names.append(name)
            out_avals.append(jax.core.ShapedArray(
                tuple(alloc.tensor_shape), mybir.dt.np(alloc.dtype)))
    bind_names = list(in_names) + list(out_names)
    if part_name is not None:
        bind_names.append(part_name)
    bind_names = tuple(bind_names)
    n_in = len(in_names)

    def _body(*args):
        operands = list(args)
        if part_name is not None:
            operands.append(bass2jax.partition_id_tensor())
        outs = bass2jax._bass_exec_p.bind(
            *operands, out_avals=tuple(out_avals), in_names=bind_names,
            out_names=tuple(out_names), lowering_input_output_aliases=(),
            sim_require_finite=True, sim_require_nnan=True, nc=nc)
        return tuple(outs)

    mesh = Mesh(np.asarray(jax.devices()[:NCORES]), ("core",))
    spec = PartitionSpec("core")
    nsh = NamedSharding(mesh, spec)
    n_out = len(out_names)
    fn = jax.jit(
        shard_map(_body, mesh=mesh, in_specs=(spec,) * (n_in + n_out),
                  out_specs=(spec,) * n_out, check_rep=False),
        donate_argnums=tuple(range(n_in, n_in + n_out)), keep_unused=True)
    r = (fn, in_names, out_names, out_avals, nsh)
    _CACHE[key] = r
    return r


def _stage(name, arr, nsh, replicate=True):
    """device_put a per-core-replicated (or already stacked) array, cached."""
    import jax
    key = ("dev", name)
    if key not in _CACHE:
        big = np.concatenate([arr] * NCORES, axis=0) if replicate else arr
        _CACHE[key] = jax.device_put(big, nsh)
    return _CACHE[key]


def _unpack_static(spec):
    if len(spec) == 3:
        return spec
    arr, ck = spec
    return arr, ck, True


def _zeros(shape, dtype, nsh):
    import jax, jax.numpy as jnp
    key = ("zfn", shape, str(dtype))
    if key not in _CACHE:
        _CACHE[key] = jax.jit(lambda: jnp.zeros(shape, dtype),
                              out_shardings=nsh)
    return _CACHE[key]()


def _run_fast(nc, tag, dyn_inputs, static_inputs):
    """dyn_inputs: name -> stacked np/jax array [8*d0, ...] (per-call);
    static_inputs: name -> (per-core np array, cache_key) staged once."""
    import jax
    fn, in_names, out_names, out_avals, nsh = _runner(nc, tag)
    args = []
    for name in in_names:
        if name in dyn_inputs:
            v = dyn_inputs[name]
            if isinstance(v, np.ndarray):
                v = jax.device_put(v, nsh)
            args.append(v)
        else:
            arr, ck, rep = _unpack_static(static_inputs[name])
            args.append(_stage(ck, arr, nsh, replicate=rep))
    for av in out_avals:
        args.append(_zeros((NCORES * av.shape[0],) + av.shape[1:], av.dtype, nsh))
    outs = fn(*args)
    return dict(zip(out_names, outs))


# ---------- traced path (timing) ----------

def _run_traced(nc, in_maps, label):
    _install_shim()
    res = run_bass_kernel_spmd(nc, in_maps, core_ids=list(range(NCORES)),
                               trace=True)
    if res.exec_time_ns is not None:
        LAST_EXEC_NS.append((label, res.exec_time_ns))
    return res.results


def kernel(x, emb, wq, wk, wv, wo, ln1_g, ln1_b, w1, b1, w2, b2, ln2_g, ln2_b,
           fo_w, fo_b):
    del LAST_EXEC_NS[:]
    x = np.asarray(x)
    f32 = lambda a: np.ascontiguousarray(np.asarray(a, np.float32))
    emb = f32(emb)
    wq, wk, wv, wo = f32(wq), f32(wk), f32(wv), f32(wo)
    w1, w2, b1, b2 = f32(w1), f32(w2), f32(b1), f32(b2)
    ln1_g, ln1_b, ln2_g, ln2_b = f32(ln1_g), f32(ln1_b), f32(ln2_g), f32(ln2_b)
    fo_w, fo_b = f32(fo_w), f32(fo_b)

    ncA, ncB = _programs()
    C2, S2, ROT = host_consts()

    h0 = emb[x.astype(np.int64)]  # [16, 128, 1024]
    hT0 = np.concatenate(
        [np.ascontiguousarray(h0[2 * c:2 * c + 2].reshape(T, D).T)
         for c in range(NCORES)], axis=0)  # [8*1024, 256]

    wsig = float(np.float64(wq[0, 0, 0]))  # cache buster across weight sets

    def a_static(lo):
        key = ("apack", lo, wsig)
        if key not in _CACHE:
            sl = slice(lo, lo + 3)
            _CACHE[key] = {
                'wq': _pack_qkvo(wq[sl], ln1_g[sl]),
                'wk': _pack_qkvo(wk[sl], ln1_g[sl]),
                'wv': _pack_qkvo(wv[sl], ln1_g[sl]),
                'wo': _pack_qkvo(wo[sl], np.ones_like(ln1_g[sl])),
                'w1': _pack_w1(w1[sl], ln2_g[sl]),
                'w2': _pack_w2(w2[sl]),
                'pb': _pack_pb(b1[sl], ln2_b[sl], w1[sl], b2[sl]),
                'sb': _pack_sb(ln1_b[sl], wq[sl], wk[sl], wv[sl]),
                'C2': C2, 'S2': S2, 'ROT': ROT,
            }
        return {k: (v, (k, lo, wsig)) for k, v in _CACHE[key].items()}

    use_traced = _trace_on()

    def runA(hT_stacked, lo, label):
        if use_traced:
            hTs = np.asarray(hT_stacked).reshape(NCORES, D, T)
            com = {k: v for k, (v, _) in a_static(lo).items()}
            maps = [{**com, 'hT_in': hTs[c]} for c in range(NCORES)]
            r = _run_traced(ncA, maps, label)
            hT_next = np.concatenate([r[c]['hT_out'] for c in range(NCORES)])
            ents = np.stack([r[c]['ent'] for c in range(NCORES)])
            return hT_next, ents
        out = _run_fast(ncA, "A", {'hT_in': hT_stacked}, a_static(lo))
        ents = np.asarray(out['ent']).reshape(NCORES, 3, 128, 2 * 16)
        return out['hT_out'], ents

    hT1, ent1 = runA(hT0, 0, "A1")
    e = ent1.reshape(NCORES, 3, S, 2, 16).transpose(1, 0, 3, 2, 4)
    e = e.reshape(3, B * S, 16).astype(np.float32)
    g = np.mean([np.var(e[l], axis=-1, ddof=1).mean() for l in range(3)])

    if g < 0.6:
        hT2, _ = runA(hT1, 0, "A2")
    else:
        hT2 = hT1
    hT3, _ = runA(hT2, 3, "A3")

    # ---------- fc_out (vocab-sharded) ----------
    hTfull = _pack_hT_full(hT3)                      # [128, KT, 2048] bf16
    fo_pack = _pack_fo(fo_w)                         # [64, 128, KT, 500]
    fob_pack = np.ascontiguousarray(
        fo_b.reshape(NCORES, 1, VS).astype(BF_NP))   # [8, 1, 4000]

    if use_traced:
        maps = [{'hT_in': hTfull,
                 'fo_w': fo_pack[c * 8:(c + 1) * 8],
                 'fo_b': fob_pack[c]}
                for c in range(NCORES)]
        rb = _run_traced(ncB, maps, "B")
        logits = np.stack([rb[c]['logits'] for c in range(NCORES)])
    else:
        hT_big = np.concatenate([hTfull] * NCORES, axis=0)
        outb = _run_fast(ncB, "B", {'hT_in': hT_big},
                         {'fo_w': (fo_pack, ('fo_w', wsig), False),
                          'fo_b': (fob_pack.reshape(NCORES * 1, VS),
                                   ('fo_b', wsig), False)})
        logits = np.asarray(outb['logits']).reshape(NCORES, TFULL, VS)

    out = np.empty((B, S, V), np.float32)
    for c in range(NCORES):
        out[:, :, c * VS:(c + 1) * VS] = logits[c].reshape(B, S, VS)
    return out
